# revision 43
# baseline (speedup 1.0000x reference)
"""MoE SwiGLU feed-forward (top-2, E=8) on 8 trn2 cores — exact-capacity EP.

Expert parallelism (core e = expert e). Host routes tokens (fp64 gating),
groups them per expert, and pads only to a multiple of 8 tokens. Per core:
  B: H[i, t] = silu(x W1^T) * ((w*x) W2^T)   fp16 matmuls, fp32 PSUM
  C: out[d, t] = sum_i H[i, t] W3[d, i]      W3 stationary, H moving
Host scatter-adds the two expert contributions per token.

vs a 128-token-padded baseline (375 us -> ~359 us):
  - Capacity C = max_e n_e rounded to 8, not 128: phase B streams arbitrary
    token chunks, and phase C with W3 stationary costs tokens, not tiles.
    Output lands as [D, C] fp16 (d on partitions); the host transposes.
  - The per-token gate weight is folded into the MM2 operand on the host
    (x2 = x * w; that branch is linear in x), so no on-device scaling.
  - Head: warm-tile memsets on the idle GpSimd engine, 8 warmup matmuls to
    ramp the PE p-state while the first DMAs land, and a 3-wide first i-tile
    pass so the PE's fresh-byte demand stays under the DMA rate.

Hardcoded: x [4,2048,1024], Wg [8,1024], W1/W2 [8,2048,1024], W3 [8,1024,2048].
"""

import numpy as np

P = 128
D = 1024
I = 2048
E = 8
TOP_K = 2
N_CORES = 8
KD = D // P  # 8
KI = I // P  # 16
ND = D // P  # 8 output d-tiles

_BUILD_CACHE: dict[int, object] = {}
LAST_RESULTS = None


def _chunks_of(C, lead=None):
    sizes = []
    if lead and C > lead:
        sizes.append(lead)
        C -= lead
    sizes += [512] * (C // 512)
    if C % 512:
        sizes.append(C % 512)
    out, off = [], 0
    for s in sizes:
        out.append((off, s))
        off += s
    return out


def _build_nc(C: int):
    import concourse.bass as bass  # noqa: F401
    import concourse.mybir as mybir
    import concourse.tile as tile
    from concourse import bacc

    fp16 = mybir.dt.float16
    fp32 = mybir.dt.float32
    SILU = mybir.ActivationFunctionType.Silu

    nc = bacc.Bacc(
        "TRN2",
        target_bir_lowering=False,
        debug=False,
        enable_asserts=False,
        num_devices=N_CORES,
    )

    # DRAM I/O (host-pre-tiled, contiguous per partition):
    #   xT/x2 [P, KD, C]   xT[p, kd, t] = x[t, kd*P+p];  x2 = x * gate_weight
    #   w1t/w2t [KI, P, KD, P]  [it][p, kd, c] = W[it*P+c, kd*P+p]
    #   w3t [P, KI, D]     w3t[p, ki, d] = W3[d, ki*P + p]
    #   out [D, C] fp16    out[d, t] (host transposes back)
    xT = nc.dram_tensor("xT", [P, KD, C], fp16, kind="ExternalInput")
    x2 = nc.dram_tensor("x2", [P, KD, C], fp16, kind="ExternalInput")
    w1t = nc.dram_tensor("w1t", [KI, P, KD, P], fp16, kind="ExternalInput")
    w2t = nc.dram_tensor("w2t", [KI, P, KD, P], fp16, kind="ExternalInput")
    w3t = nc.dram_tensor("w3t", [P, KI, D], fp16, kind="ExternalInput")
    out = nc.dram_tensor("out", [D, C], fp16, kind="ExternalOutput")

    chunks = _chunks_of(C, lead=256)

    with tile.TileContext(nc) as tc:
        with (
            tc.tile_pool(name="resident", bufs=1) as res,
            tc.tile_pool(name="wstream", bufs=4) as wpool,
            tc.tile_pool(name="tmp", bufs=4) as tmp,
            tc.tile_pool(name="outp", bufs=4) as outp,
            tc.tile_pool(name="ps1", bufs=2, space="PSUM") as ps1,
            tc.tile_pool(name="ps2", bufs=2, space="PSUM") as ps2,
            tc.tile_pool(name="ps3", bufs=4, space="PSUM") as ps3,
        ):
            xT_s = res.tile([P, KD, C], fp16)
            x2_s = res.tile([P, KD, C], fp16)
            H = res.tile([P, KI, C], fp16)
            w3_s = res.tile([P, KI, D], fp16)

            # PE p-state warm + Silu table preload during the head DMA wait.
            warm_a = res.tile([P, P], fp16)
            warm_b = res.tile([P, 512], fp16)
            nc.gpsimd.memset(warm_a[:], 0.0)
            nc.gpsimd.memset(warm_b[:], 0.0)
            wps = ps3.tile([P, 512], fp32, tag="po")
            for _ in range(8):
                nc.tensor.matmul(
                    wps[:], warm_a[:], warm_b[:], start=True, stop=True
                )
            act_warm = tmp.tile([P, 1], fp16, tag="actw")
            nc.scalar.activation(act_warm[:], warm_a[:, :1], SILU)

            # Head DMAs: first xT chunk + first weight pair, then x2 chunk,
            # then the rest.
            t0, tw = chunks[0]
            nc.sync.dma_start(xT_s[:, :, t0 : t0 + tw], xT[:, :, t0 : t0 + tw])

            w_tiles = {}

            def get_w(it):
                if it not in w_tiles:
                    a = wpool.tile([P, KD, P], fp16, tag="w1")
                    b = wpool.tile([P, KD, P], fp16, tag="w2")
                    nc.sync.dma_start(a[:], w1t[it, :, :, :])
                    nc.sync.dma_start(b[:], w2t[it, :, :, :])
                    w_tiles[it] = (a, b)
                return w_tiles[it]

            get_w(0)
            nc.sync.dma_start(x2_s[:, :, t0 : t0 + tw], x2[:, :, t0 : t0 + tw])
            get_w(1)
            get_w(2)
            get_w(3)
            for t0, tw in chunks[1:]:
                nc.sync.dma_start(xT_s[:, :, t0 : t0 + tw], xT[:, :, t0 : t0 + tw])
                nc.sync.dma_start(x2_s[:, :, t0 : t0 + tw], x2[:, :, t0 : t0 + tw])

            # Phase B schedule: first three i-tiles interleave chunk-by-chunk
            # (cuts the head DMA rate the PE needs), then i-tile major.
            sched = []
            for c in chunks:
                for it in (0, 1, 2, 3):
                    sched.append((it, c))
            for it in range(4, KI):
                for c in chunks:
                    sched.append((it, c))

            w3_at = min(len(sched) - 1, 4 * len(chunks) + 10)
            for si, (it, (t0, tw)) in enumerate(sched):
                w1_s, w2_s = get_w(it)
                p1 = ps1.tile([P, 512], fp32)
                p2 = ps2.tile([P, 512], fp32)
                for kd in range(KD):
                    nc.tensor.matmul(
                        p1[:, :tw],
                        w1_s[:, kd, :],
                        xT_s[:, kd, t0 : t0 + tw],
                        start=(kd == 0),
                        stop=(kd == KD - 1),
                    )
                for kd in range(KD):
                    nc.tensor.matmul(
                        p2[:, :tw],
                        w2_s[:, kd, :],
                        x2_s[:, kd, t0 : t0 + tw],
                        start=(kd == 0),
                        stop=(kd == KD - 1),
                    )
                sil = tmp.tile([P, 512], fp16)
                nc.scalar.activation(sil[:, :tw], p1[:, :tw], SILU)
                nc.vector.tensor_mul(
                    H[:, it, t0 : t0 + tw], sil[:, :tw], p2[:, :tw]
                )
                if si == w3_at:
                    nc.sync.dma_start(w3_s[:], w3t[:])

            # Phase C: out[d, t] = sum_i H[i, t] W3[d, i] — W3 tile stationary,
            # H moving, cost proportional to tokens. Copy PSUM->SBUF alternates
            # scalar/vector; DMA straight out per (d-tile, chunk).
            for t0, tw in chunks:
                for dt in range(ND):
                    po = ps3.tile([P, 512], fp32, tag="po")
                    dsl = slice(dt * P, (dt + 1) * P)
                    for ki in range(KI):
                        nc.tensor.matmul(
                            po[:, :tw],
                            w3_s[:, ki, dsl],
                            H[:, ki, t0 : t0 + tw],
                            start=(ki == 0),
                            stop=(ki == KI - 1),
                        )
                    ot = outp.tile([P, 512], fp16)
                    if dt % 2 == 0:
                        nc.scalar.copy(ot[:, :tw], po[:, :tw])
                    else:
                        nc.vector.tensor_scalar_add(ot[:, :tw], po[:, :tw], 0.0)
                    nc.sync.dma_start(out[dsl, t0 : t0 + tw], ot[:, :tw])

    nc.compile()
    return nc


def _route(xf64: np.ndarray, Wg64: np.ndarray):
    """Top-2 routing in fp64 (selection matches jax fp32 on this dataset)."""
    scores = xf64 @ Wg64.T
    order = np.argsort(-scores, axis=1, kind="stable")[:, :TOP_K]
    s1 = np.take_along_axis(scores, order, axis=1)
    e2 = np.exp(s1[:, 1] - s1[:, 0])
    p1 = 1.0 / (1.0 + e2)
    pw = np.stack([p1, 1.0 - p1], axis=1)
    idx_list, w_list = [], []
    for e in range(E):
        mask = order == e
        tok = np.nonzero(mask.any(axis=1))[0]
        wv = (pw * mask)[tok].sum(axis=1)
        idx_list.append(tok)
        w_list.append(wv.astype(np.float32))
    return idx_list, w_list


def kernel(x, Wg, W1, W2, W3):
    global LAST_RESULTS
    from concourse.bass_utils import run_bass_kernel_spmd

    x = np.asarray(x, dtype=np.float32)
    Wg = np.asarray(Wg, dtype=np.float32)
    W1 = np.asarray(W1, dtype=np.float32)
    W2 = np.asarray(W2, dtype=np.float32)
    W3 = np.asarray(W3, dtype=np.float32)

    B, S, _ = x.shape
    T = B * S
    xf = x.reshape(T, D)

    idx_list, w_list = _route(xf.astype(np.float64), Wg.astype(np.float64))
    C = max(len(t) for t in idx_list)
    C = ((C + 7) // 8) * 8

    if C not in _BUILD_CACHE:
        _BUILD_CACHE[C] = _build_nc(C)
    nc = _BUILD_CACHE[C]

    in_maps = []
    for e in range(E):
        tok, wv = idx_list[e], w_list[e]
        n = len(tok)

        xe = np.zeros((C, D), dtype=np.float16)
        xe[:n] = xf[tok]
        xTP = np.ascontiguousarray(xe.T.reshape(KD, P, C).transpose(1, 0, 2))

        x2e = np.zeros((C, D), dtype=np.float16)
        x2e[:n] = xf[tok] * wv[:, None]
        x2P = np.ascontiguousarray(x2e.T.reshape(KD, P, C).transpose(1, 0, 2))

        w1P = np.ascontiguousarray(
            W1[e].reshape(KI, P, KD, P).transpose(0, 3, 2, 1).astype(np.float16)
        )
        w2P = np.ascontiguousarray(
            W2[e].reshape(KI, P, KD, P).transpose(0, 3, 2, 1).astype(np.float16)
        )
        w3P = np.ascontiguousarray(
            W3[e].reshape(D, KI, P).transpose(2, 1, 0).astype(np.float16)
        )

        in_maps.append({"xT": xTP, "x2": x2P, "w1t": w1P, "w2t": w2P, "w3t": w3P})

    LAST_RESULTS = run_bass_kernel_spmd(nc, in_maps, core_ids=list(range(N_CORES)))

    outf = np.zeros((T, D), dtype=np.float32)
    for e in range(E):
        y = LAST_RESULTS.results[e]["out"]  # [D, C] fp16
        n = len(idx_list[e])
        outf[idx_list[e]] += y[:, :n].T.astype(np.float32)
    return outf.reshape(B, S, D)


# revision 44
# speedup vs baseline: 1.0099x; 1.0099x over previous
"""MoE SwiGLU feed-forward (top-2, E=8) on 8 trn2 cores — exact-capacity EP.

Expert parallelism (core e = expert e). Host routes tokens (fp64 gating),
groups them per expert, and pads only to a multiple of 8 tokens. Per core:
  B: H[i, t] = silu(x W1^T) * ((w*x) W2^T)   fp16 matmuls, fp32 PSUM
  C: out[d, t] = sum_i H[i, t] W3[d, i]      W3 stationary, H moving
Host scatter-adds the two expert contributions per token.

vs a 128-token-padded baseline (375 us -> ~359 us):
  - Capacity C = max_e n_e rounded to 8, not 128: phase B streams arbitrary
    token chunks, and phase C with W3 stationary costs tokens, not tiles.
    Output lands as [D, C] fp16 (d on partitions); the host transposes.
  - The per-token gate weight is folded into the MM2 operand on the host
    (x2 = x * w; that branch is linear in x), so no on-device scaling.
  - Head: warm-tile memsets on the idle GpSimd engine, 8 warmup matmuls to
    ramp the PE p-state while the first DMAs land, and a 3-wide first i-tile
    pass so the PE's fresh-byte demand stays under the DMA rate.

Hardcoded: x [4,2048,1024], Wg [8,1024], W1/W2 [8,2048,1024], W3 [8,1024,2048].
"""

import numpy as np

P = 128
D = 1024
I = 2048
E = 8
TOP_K = 2
N_CORES = 8
KD = D // P  # 8
KI = I // P  # 16
ND = D // P  # 8 output d-tiles

_BUILD_CACHE: dict[int, object] = {}
LAST_RESULTS = None


def _chunks_of(C, lead=None):
    sizes = []
    if lead and C > lead:
        sizes.append(lead)
        C -= lead
    sizes += [512] * (C // 512)
    if C % 512:
        sizes.append(C % 512)
    out, off = [], 0
    for s in sizes:
        out.append((off, s))
        off += s
    return out


def _build_nc(C: int):
    import concourse.bass as bass  # noqa: F401
    import concourse.mybir as mybir
    import concourse.tile as tile
    from concourse import bacc

    fp16 = mybir.dt.float16
    fp32 = mybir.dt.float32
    SILU = mybir.ActivationFunctionType.Silu

    nc = bacc.Bacc(
        "TRN2",
        target_bir_lowering=False,
        debug=False,
        enable_asserts=False,
        num_devices=N_CORES,
    )

    # DRAM I/O (host-pre-tiled, contiguous per partition):
    #   xT/x2 [P, KD, C]   xT[p, kd, t] = x[t, kd*P+p];  x2 = x * gate_weight
    #   w1t/w2t [KI, P, KD, P]  [it][p, kd, c] = W[it*P+c, kd*P+p]
    #   w3t [P, KI, D]     w3t[p, ki, d] = W3[d, ki*P + p]
    #   out [D, C] fp16    out[d, t] (host transposes back)
    xT = nc.dram_tensor("xT", [P, KD, C], fp16, kind="ExternalInput")
    x2 = nc.dram_tensor("x2", [P, KD, C], fp16, kind="ExternalInput")
    w1t = nc.dram_tensor("w1t", [KI, P, KD, P], fp16, kind="ExternalInput")
    w2t = nc.dram_tensor("w2t", [KI, P, KD, P], fp16, kind="ExternalInput")
    w3t = nc.dram_tensor("w3t", [P, KI, D], fp16, kind="ExternalInput")
    out = nc.dram_tensor("out", [D, C], fp16, kind="ExternalOutput")

    chunks = _chunks_of(C, lead=256)

    with tile.TileContext(nc) as tc:
        with (
            tc.tile_pool(name="resident", bufs=1) as res,
            tc.tile_pool(name="wstream", bufs=3) as wpool,
            tc.tile_pool(name="tmp", bufs=4) as tmp,
            tc.tile_pool(name="outp", bufs=4) as outp,
            tc.tile_pool(name="ps1", bufs=2, space="PSUM") as ps1,
            tc.tile_pool(name="ps2", bufs=2, space="PSUM") as ps2,
            tc.tile_pool(name="ps3", bufs=4, space="PSUM") as ps3,
        ):
            xT_s = res.tile([P, KD, C], fp16)
            x2_s = res.tile([P, KD, C], fp16)
            H = res.tile([P, KI, C], fp16)
            w3_s = res.tile([P, KI, D], fp16)

            # PE p-state warm + Silu table preload during the head DMA wait.
            warm_a = res.tile([P, P], fp16)
            warm_b = res.tile([P, 512], fp16)
            nc.gpsimd.memset(warm_a[:], 0.0)
            nc.gpsimd.memset(warm_b[:], 0.0)
            wps = ps3.tile([P, 512], fp32, tag="po")
            for _ in range(8):
                nc.tensor.matmul(
                    wps[:], warm_a[:], warm_b[:], start=True, stop=True
                )
            act_warm = tmp.tile([P, 1], fp16, tag="actw")
            nc.scalar.activation(act_warm[:], warm_a[:, :1], SILU)

            # Head DMAs: first xT chunk + first weight pair, then x2 chunk,
            # then the rest.
            t0, tw = chunks[0]
            nc.sync.dma_start(xT_s[:, :, t0 : t0 + tw], xT[:, :, t0 : t0 + tw])

            w_tiles = {}

            def get_w(it):
                if it not in w_tiles:
                    a = wpool.tile([P, KD, P], fp16, tag="w1")
                    b = wpool.tile([P, KD, P], fp16, tag="w2")
                    nc.sync.dma_start(a[:], w1t[it, :, :, :])
                    nc.sync.dma_start(b[:], w2t[it, :, :, :])
                    w_tiles[it] = (a, b)
                return w_tiles[it]

            get_w(0)
            nc.sync.dma_start(x2_s[:, :, t0 : t0 + tw], x2[:, :, t0 : t0 + tw])
            get_w(1)
            get_w(2)
            for t0, tw in chunks[1:]:
                nc.sync.dma_start(xT_s[:, :, t0 : t0 + tw], xT[:, :, t0 : t0 + tw])
                nc.sync.dma_start(x2_s[:, :, t0 : t0 + tw], x2[:, :, t0 : t0 + tw])

            # Phase B schedule: first three i-tiles interleave chunk-by-chunk
            # (cuts the head DMA rate the PE needs), then i-tile major.
            sched = []
            for c in chunks:
                for it in (0, 1, 2):
                    sched.append((it, c))
            for it in range(3, KI):
                for c in chunks:
                    sched.append((it, c))

            w3_at = min(len(sched) - 1, 3 * len(chunks) + 10)
            for si, (it, (t0, tw)) in enumerate(sched):
                w1_s, w2_s = get_w(it)
                p1 = ps1.tile([P, 512], fp32)
                p2 = ps2.tile([P, 512], fp32)
                for kd in range(KD):
                    nc.tensor.matmul(
                        p1[:, :tw],
                        w1_s[:, kd, :],
                        xT_s[:, kd, t0 : t0 + tw],
                        start=(kd == 0),
                        stop=(kd == KD - 1),
                    )
                for kd in range(KD):
                    nc.tensor.matmul(
                        p2[:, :tw],
                        w2_s[:, kd, :],
                        x2_s[:, kd, t0 : t0 + tw],
                        start=(kd == 0),
                        stop=(kd == KD - 1),
                    )
                sil = tmp.tile([P, 512], fp16)
                nc.scalar.activation(sil[:, :tw], p1[:, :tw], SILU)
                nc.vector.tensor_mul(
                    H[:, it, t0 : t0 + tw], sil[:, :tw], p2[:, :tw]
                )
                if si == w3_at:
                    nc.sync.dma_start(w3_s[:], w3t[:])

            # Phase C: out[d, t] = sum_i H[i, t] W3[d, i] — W3 tile stationary,
            # H moving, cost proportional to tokens. Copy PSUM->SBUF alternates
            # scalar/vector; DMA straight out per (d-tile, chunk).
            for t0, tw in chunks:
                for dt in range(ND):
                    po = ps3.tile([P, 512], fp32, tag="po")
                    dsl = slice(dt * P, (dt + 1) * P)
                    for ki in range(KI):
                        nc.tensor.matmul(
                            po[:, :tw],
                            w3_s[:, ki, dsl],
                            H[:, ki, t0 : t0 + tw],
                            start=(ki == 0),
                            stop=(ki == KI - 1),
                        )
                    ot = outp.tile([P, 512], fp16)
                    if dt % 2 == 0:
                        nc.scalar.copy(ot[:, :tw], po[:, :tw])
                    else:
                        nc.vector.tensor_scalar_add(ot[:, :tw], po[:, :tw], 0.0)
                    nc.sync.dma_start(out[dsl, t0 : t0 + tw], ot[:, :tw])

    nc.compile()
    return nc


def _route(xf64: np.ndarray, Wg64: np.ndarray):
    """Top-2 routing in fp64 (selection matches jax fp32 on this dataset)."""
    scores = xf64 @ Wg64.T
    order = np.argsort(-scores, axis=1, kind="stable")[:, :TOP_K]
    s1 = np.take_along_axis(scores, order, axis=1)
    e2 = np.exp(s1[:, 1] - s1[:, 0])
    p1 = 1.0 / (1.0 + e2)
    pw = np.stack([p1, 1.0 - p1], axis=1)
    idx_list, w_list = [], []
    for e in range(E):
        mask = order == e
        tok = np.nonzero(mask.any(axis=1))[0]
        wv = (pw * mask)[tok].sum(axis=1)
        idx_list.append(tok)
        w_list.append(wv.astype(np.float32))
    return idx_list, w_list


def kernel(x, Wg, W1, W2, W3):
    global LAST_RESULTS
    from concourse.bass_utils import run_bass_kernel_spmd

    x = np.asarray(x, dtype=np.float32)
    Wg = np.asarray(Wg, dtype=np.float32)
    W1 = np.asarray(W1, dtype=np.float32)
    W2 = np.asarray(W2, dtype=np.float32)
    W3 = np.asarray(W3, dtype=np.float32)

    B, S, _ = x.shape
    T = B * S
    xf = x.reshape(T, D)

    idx_list, w_list = _route(xf.astype(np.float64), Wg.astype(np.float64))
    C = max(len(t) for t in idx_list)
    C = ((C + 7) // 8) * 8

    if C not in _BUILD_CACHE:
        _BUILD_CACHE[C] = _build_nc(C)
    nc = _BUILD_CACHE[C]

    in_maps = []
    for e in range(E):
        tok, wv = idx_list[e], w_list[e]
        n = len(tok)

        xe = np.zeros((C, D), dtype=np.float16)
        xe[:n] = xf[tok]
        xTP = np.ascontiguousarray(xe.T.reshape(KD, P, C).transpose(1, 0, 2))

        x2e = np.zeros((C, D), dtype=np.float16)
        x2e[:n] = xf[tok] * wv[:, None]
        x2P = np.ascontiguousarray(x2e.T.reshape(KD, P, C).transpose(1, 0, 2))

        w1P = np.ascontiguousarray(
            W1[e].reshape(KI, P, KD, P).transpose(0, 3, 2, 1).astype(np.float16)
        )
        w2P = np.ascontiguousarray(
            W2[e].reshape(KI, P, KD, P).transpose(0, 3, 2, 1).astype(np.float16)
        )
        w3P = np.ascontiguousarray(
            W3[e].reshape(D, KI, P).transpose(2, 1, 0).astype(np.float16)
        )

        in_maps.append({"xT": xTP, "x2": x2P, "w1t": w1P, "w2t": w2P, "w3t": w3P})

    LAST_RESULTS = run_bass_kernel_spmd(nc, in_maps, core_ids=list(range(N_CORES)))

    outf = np.zeros((T, D), dtype=np.float32)
    for e in range(E):
        y = LAST_RESULTS.results[e]["out"]  # [D, C] fp16
        n = len(idx_list[e])
        outf[idx_list[e]] += y[:, :n].T.astype(np.float32)
    return outf.reshape(B, S, D)
